# revision 1
# baseline (speedup 1.0000x reference)
"""MoE (top-2 of 8 experts, d=1024) — expert-parallel Bass kernel for 8 trn2 cores.

Strategy (per sharding_hint "Expert-parallel"): shard W1/W2/b1/b2 along the
expert axis (expert e -> core e). The host computes the gate scores and top-2
assignment (0.2% of model FLOPs, deterministic) to build the dispatch: each
core receives exactly the tokens routed to its expert (padded to capacity C,
transposed and chunk-major so every HBM block streams sequentially). Each core
computes   yT = (relu(W1^T xT + b1)^T W2 + b2) * w   with float32r matmuls
(full PE rate, ~1e-4 accuracy); the host scatter-adds the two expert
contributions per token (the "combine" of the return all-to-all).

Device-side details:
 - per-kc split DMAs so the first matmul waits on 0.75MB, not 10MB
 - combine weights broadcast across partitions on-device (K=1 ones matmul,
   all chunks upfront) so no per-token broadcast traffic from HBM
 - bias-add + relu fused into single DVE tensor_scalar ops reading PSUM
   (keeps ScalarE off the critical path)
 - chunk sizes [512]*nb + [tail] with C a multiple of 256 (f32r keeps
   1 cycle/row down to a free dim of 256)
"""

import numpy as np

import concourse.bass as bass
import concourse.mybir as mybir
import concourse.tile as tile
from concourse import bacc
from concourse.bass_utils import run_bass_kernel_spmd

# Problem shapes (hardcoded per contract)
D = 1024  # d_model == d_hidden
N_EXPERTS = 8
TOP_K = 2
N_CORES = 8
B, T = 4, 2048
N_TOKENS = B * T

F32 = mybir.dt.float32
F32R = mybir.dt.float32r
KC = D // 128  # contraction chunks (8)
MC = D // 128  # output-feature chunks (8)
NT = 512      # tokens per matmul (moving free dim; fp32 max)
CGRAIN = 256  # capacity granularity (f32r needs free dim >= 256 for full rate)


def chunk_sizes(C):
    assert C % CGRAIN == 0
    sizes = [NT] * (C // NT)
    if C % NT:
        sizes.append(C % NT)
    return sizes


def build_moe_expert_kernel(C: int, repeat: int = 1, split_w: int = 8,
                            split_x: bool = True, split_y: bool = True,
                            pipe: bool = False, stagger: bool = False,
                            dve_elt: bool = True,
                            wb_all: bool = True) -> bacc.Bacc:
    """One-expert MLP kernel: yT = (relu(x@W1+b1)@W2 + b2) * w, chunk-major.

    DRAM inputs: xTb [nb, D, NT] (+ xTt [D, tail] if C%NT), wvec [1, C],
    ones [1, 128], w1 [D, D], b1 [D], w2 [D, D], b2 [D].
    Outputs: yTb [nb, D, NT] (+ yTt [D, tail]).
    `repeat` wraps the computation in a hardware loop (slope-based HW timing).
    """
    sizes = chunk_sizes(C)
    nb = sum(1 for s in sizes if s == NT)
    tail = C % NT

    nc = bacc.Bacc("TRN2", target_bir_lowering=False, debug=False,
                   num_devices=N_CORES)

    xTb = nc.dram_tensor("xTb", [nb, D, NT], F32R, kind="ExternalInput")
    wvec = nc.dram_tensor("wvec", [1, C], F32R, kind="ExternalInput")
    ones = nc.dram_tensor("ones", [1, 128], F32R, kind="ExternalInput")
    w1 = nc.dram_tensor("w1", [D, D], F32R, kind="ExternalInput")
    b1 = nc.dram_tensor("b1", [D], F32, kind="ExternalInput")
    w2 = nc.dram_tensor("w2", [D, D], F32R, kind="ExternalInput")
    b2 = nc.dram_tensor("b2", [D], F32, kind="ExternalInput")
    yTb = nc.dram_tensor("yTb", [nb, D, NT], F32, kind="ExternalOutput")
    if tail:
        xTt = nc.dram_tensor("xTt", [D, tail], F32R, kind="ExternalInput")
        yTt = nc.dram_tensor("yTt", [D, tail], F32, kind="ExternalOutput")

    # DRAM views: partition-dim-first tilings (chunk blocks are contiguous)
    xTb_v = xTb.ap().rearrange("n (kc kp) t -> n kp kc t", kc=KC)
    w1_v = w1.ap().rearrange("(kc kp) m -> kp kc m", kc=KC)      # [128, KC, D]
    w2_v = w2.ap().rearrange("(kc kp) m -> kp kc m", kc=KC)
    b1_v = b1.ap().rearrange("(mc mp) -> mp mc", mc=MC)          # [128, MC]
    b2_v = b2.ap().rearrange("(mc mp) -> mp mc", mc=MC)
    yTb_v = yTb.ap().rearrange("n (mc mp) t -> n mp mc t", mc=MC)
    if tail:
        xTt_v = xTt.ap().rearrange("(kc kp) t -> kp kc t", kc=KC)
        yTt_v = yTt.ap().rearrange("(mc mp) t -> mp mc t", mc=MC)

    def x_view(n):
        return xTb_v[n] if sizes[n] == NT else xTt_v

    def y_view(n):
        return yTb_v[n] if sizes[n] == NT else yTt_v

    with tile.TileContext(nc) as tc:
        with (
            tc.tile_pool(name="weights", bufs=1) as wpool,
            tc.tile_pool(name="consts", bufs=1) as cpool,
            tc.tile_pool(name="xin", bufs=3) as xpool,
            tc.tile_pool(name="hmid", bufs=2) as hpool,
            tc.tile_pool(name="yout", bufs=2) as ypool,
            tc.tile_pool(name="wbp", bufs=2) as wbpool,
            tc.tile_pool(name="ph", bufs=3, space="PSUM") as phpool,
            tc.tile_pool(name="py", bufs=3, space="PSUM") as pypool,
            tc.tile_pool(name="pw", bufs=2, space="PSUM") as pwpool,
        ):
            from contextlib import nullcontext
            loop_cm = (
                tc.For_i(0, repeat, 1,
                         hint_engines=(mybir.EngineType.PE,
                                       mybir.EngineType.Activation,
                                       mybir.EngineType.DVE,
                                       mybir.EngineType.SP),
                         staggered_reset=stagger)
                if repeat > 1 else nullcontext()
            )
            with loop_cm:
                # Per-kc split DMAs: the first matmul only waits for its own
                # 512KB weight slice + 256KB x slice instead of the whole
                # prologue (model: first MM 36.6us -> 5.1us).
                w1_sb = wpool.tile([128, KC, D], F32R, tag="w1")
                w2_sb = wpool.tile([128, KC, D], F32R, tag="w2")
                b1_sb = cpool.tile([128, MC], F32, tag="b1")
                b2_sb = cpool.tile([128, MC], F32, tag="b2")
                wv_sb = cpool.tile([1, C], F32R, tag="wv")
                on_sb = cpool.tile([1, 128], F32R, tag="ones")
                x0 = xpool.tile([128, KC, NT], F32R, tag="x")
                nc.sync.dma_start(on_sb[:], ones.ap())
                nc.sync.dma_start(wv_sb[:], wvec.ap())
                g = KC // split_w
                for i in range(split_w):
                    ks = slice(i * g, (i + 1) * g)
                    nc.sync.dma_start(w1_sb[:, ks, :], w1_v[:, ks, :])
                    if split_x:
                        for kc in range(i * g, (i + 1) * g):
                            nc.sync.dma_start(x0[:, kc, :sizes[0]],
                                              x_view(0)[:, kc, :])
                if not split_x:
                    nc.sync.dma_start(x0[:, :, :sizes[0]], x_view(0))
                nc.sync.dma_start(b1_sb[:], b1_v)

                offs = [sum(sizes[:i]) for i in range(len(sizes))]
                nchk = len(sizes)

                wb_full = (cpool.tile([128, C], F32, tag="wbf", name="wbf")
                           if wb_all else None)

                def emit_wb_full():
                    for n in range(nchk):
                        pwf = pwpool.tile([128, NT], F32, tag="pw")
                        nc.tensor.matmul(pwf[:, :sizes[n]], on_sb[:],
                                         wv_sb[:, offs[n]:offs[n] + sizes[n]],
                                         start=True, stop=True)
                        nc.vector.tensor_copy(
                            wb_full[:, offs[n]:offs[n] + sizes[n]],
                            pwf[:, :sizes[n]])

                def emit_x_dma(n, x_sb):
                    sz = sizes[n]
                    if split_x:
                        for kc in range(KC):
                            nc.sync.dma_start(x_sb[:, kc, :sz],
                                              x_view(n)[:, kc, :])
                    else:
                        nc.sync.dma_start(x_sb[:, :, :sz], x_view(n))

                def emit_wb(n):
                    # broadcast combine weights for chunk n: [128, sz]
                    if wb_all:
                        return wb_full[:, offs[n]:offs[n] + sizes[n]]
                    sz = sizes[n]
                    pw = pwpool.tile([128, NT], F32, tag="pw")
                    nc.tensor.matmul(pw[:, :sz], on_sb[:],
                                     wv_sb[:, offs[n]:offs[n] + sz],
                                     start=True, stop=True)
                    wb_sb = wbpool.tile([128, NT], F32, tag="wb")
                    nc.vector.tensor_copy(wb_sb[:, :sz], pw[:, :sz])
                    return wb_sb

                def emit_h_mc(n, mc, x_sb, h_sb):
                    sz = sizes[n]
                    ph = phpool.tile([128, NT], F32, tag="ph")
                    for kc in range(KC):
                        nc.tensor.matmul(
                            ph[:, :sz],
                            w1_sb[:, kc, bass.ts(mc, 128)],
                            x_sb[:, kc, :sz],
                            start=(kc == 0), stop=(kc == KC - 1),
                        )
                    # h = relu(ph + b1)
                    if dve_elt:
                        nc.vector.tensor_scalar(
                            h_sb[:, mc, :sz], ph[:, :sz],
                            b1_sb[:, mc:mc + 1], 0.0,
                            mybir.AluOpType.add, mybir.AluOpType.max,
                        )
                    else:
                        nc.scalar.activation(
                            h_sb[:, mc, :sz], ph[:, :sz],
                            mybir.ActivationFunctionType.Relu,
                            bias=b1_sb[:, mc:mc + 1],
                        )

                def emit_y_mc(n, mc, h_sb, y_sb, wb_sb):
                    sz = sizes[n]
                    py = pypool.tile([128, NT], F32, tag="py")
                    for kc in range(KC):
                        nc.tensor.matmul(
                            py[:, :sz],
                            w2_sb[:, kc, bass.ts(mc, 128)],
                            h_sb[:, kc, :sz],
                            start=(kc == 0), stop=(kc == KC - 1),
                        )
                    # y = (py + b2) * w
                    if dve_elt:
                        nc.vector.tensor_scalar(
                            y_sb[:, mc, :sz], py[:, :sz],
                            b2_sb[:, mc:mc + 1], None,
                            mybir.AluOpType.add,
                        )
                    else:
                        nc.scalar.activation(
                            y_sb[:, mc, :sz], py[:, :sz],
                            mybir.ActivationFunctionType.Identity,
                            bias=b2_sb[:, mc:mc + 1],
                        )
                    nc.vector.tensor_mul(
                        y_sb[:, mc, :sz], y_sb[:, mc, :sz],
                        wb_sb if wb_all else wb_sb[:, :sz],
                    )
                    if split_y:
                        nc.sync.dma_start(y_view(n)[:, mc, :],
                                          y_sb[:, mc, :sz])

                def emit_w2():
                    # w2 is first needed by chunk 0's y-phase (~27us in); keep
                    # its 4MB off the DMA engines while w1/x0 races the PE.
                    for i in range(split_w):
                        ks = slice(i * g, (i + 1) * g)
                        nc.sync.dma_start(w2_sb[:, ks, :], w2_v[:, ks, :])
                    nc.sync.dma_start(b2_sb[:], b2_v)

                if not pipe:
                    for n in range(nchk):
                        if n == 0:
                            x_sb = x0
                        else:
                            x_sb = xpool.tile([128, KC, NT], F32R, tag="x")
                            emit_x_dma(n, x_sb)
                        wb_sb = emit_wb(n)
                        h_sb = hpool.tile([128, KC, NT], F32R, tag="h")
                        for mc in range(MC):
                            emit_h_mc(n, mc, x_sb, h_sb)
                        if n == 0:
                            emit_w2()
                            if wb_all:
                                emit_wb_full()
                        y_sb = ypool.tile([128, MC, NT], F32, tag="y")
                        for mc in range(MC):
                            emit_y_mc(n, mc, h_sb, y_sb, wb_sb)
                        if not split_y:
                            nc.sync.dma_start(y_view(n), y_sb[:, :, :sizes[n]])
                else:
                    # software pipeline: stage s emits h-phase(s) interleaved
                    # with y-phase(s-1) at mc granularity, so the PE never
                    # waits on the relu tail of a chunk before starting the
                    # next chunk's first-layer matmuls.
                    emit_w2()
                    if wb_all:
                        emit_wb_full()
                    x_tiles = {0: x0}
                    h_tiles = {}
                    y_tiles = {}
                    wb_tiles = {0: emit_wb(0)}
                    for s in range(nchk + 1):
                        if s + 1 < nchk:  # prefetch x for next stage
                            xt = xpool.tile([128, KC, NT], F32R, tag="x")
                            emit_x_dma(s + 1, xt)
                            x_tiles[s + 1] = xt
                        if s < nchk:
                            h_tiles[s] = hpool.tile([128, KC, NT], F32R, tag="h", name=f"hs{s}")
                            if s + 1 < nchk:
                                wb_tiles[s + 1] = emit_wb(s + 1)
                        if s > 0:
                            y_tiles[s - 1] = ypool.tile([128, MC, NT], F32, tag="y", name=f"ys{s}")
                        for mc in range(MC):
                            if s < nchk:
                                emit_h_mc(s, mc, x_tiles[s], h_tiles[s])
                            if s > 0:
                                emit_y_mc(s - 1, mc, h_tiles[s - 1],
                                          y_tiles[s - 1], wb_tiles[s - 1])
                        if s > 0 and not split_y:
                            nc.sync.dma_start(y_view(s - 1),
                                              y_tiles[s - 1][:, :, :sizes[s - 1]])
                        x_tiles.pop(s - 1, None)

    nc.compile()
    return nc


_NC_CACHE: dict = {}


def _get_kernel(C: int, repeat: int = 1, **opts) -> bacc.Bacc:
    key = (C, repeat, tuple(sorted(opts.items())))
    if key not in _NC_CACHE:
        _NC_CACHE[key] = build_moe_expert_kernel(C, repeat, **opts)
    return _NC_CACHE[key]


def dispatch(x, W_gate, b_gate):
    """Host-side gate + top-2 dispatch plan. Returns (xf, ids, wts, C)."""
    xf = np.ascontiguousarray(np.asarray(x).reshape(-1, D), dtype=np.float32)
    scores = xf @ np.asarray(W_gate, np.float32) + np.asarray(b_gate, np.float32)
    # top-2 expert ids per token (order irrelevant: contributions are summed)
    top2 = np.argpartition(scores, N_EXPERTS - TOP_K, axis=1)[:, -TOP_K:]
    ids, wts = [], []
    for e in range(N_EXPERTS):
        tok = np.nonzero((top2 == e).any(axis=1))[0]
        ids.append(tok)
        wts.append(scores[tok, e])
    max_cnt = max(len(t) for t in ids)
    C = ((max_cnt + CGRAIN - 1) // CGRAIN) * CGRAIN
    return xf, ids, wts, C


def make_in_maps(parts, xf, ids, wts, C):
    """Build per-core input dicts (chunk-major xT blocks)."""
    W1, b1, W2, b2 = parts
    sizes = chunk_sizes(C)
    nb = sum(1 for s in sizes if s == NT)
    tail = C % NT
    in_maps = []
    for e in range(N_EXPERTS):
        cnt = len(ids[e])
        xTe = np.zeros((D, C), np.float32)
        xTe[:, :cnt] = xf[ids[e]].T
        xb = np.ascontiguousarray(
            xTe[:, :nb * NT].reshape(D, nb, NT).transpose(1, 0, 2))
        wv = np.zeros((1, C), np.float32)
        wv[0, :cnt] = wts[e]
        m = {
            "xTb": xb, "wvec": wv,
            "ones": np.ones((1, 128), np.float32),
            "w1": np.ascontiguousarray(W1[e]), "b1": b1[e],
            "w2": np.ascontiguousarray(W2[e]), "b2": b2[e],
        }
        if tail:
            m["xTt"] = np.ascontiguousarray(xTe[:, nb * NT:])
        in_maps.append(m)
    return in_maps


def kernel(x, W_gate, b_gate, W1, b1, W2, b2):
    xf, ids, wts, C = dispatch(x, W_gate, b_gate)
    nc = _get_kernel(C)

    W1 = np.asarray(W1, np.float32)
    W2 = np.asarray(W2, np.float32)
    b1 = np.asarray(b1, np.float32)
    b2 = np.asarray(b2, np.float32)
    in_maps = make_in_maps((W1, b1, W2, b2), xf, ids, wts, C)

    res = run_bass_kernel_spmd(nc, in_maps, core_ids=list(range(N_CORES)))

    sizes = chunk_sizes(C)
    nb = sum(1 for s in sizes if s == NT)
    tail = C % NT
    out = np.zeros((N_TOKENS, D), np.float32)
    for e in range(N_EXPERTS):
        cnt = len(ids[e])
        r = res.results[e]
        yTe = r["yTb"].transpose(1, 0, 2).reshape(D, nb * NT)
        if tail:
            yTe = np.concatenate([yTe, r["yTt"]], axis=1)
        out[ids[e]] += yTe.T[:cnt]
    return out.reshape(B, T, D)



# revision 3
# speedup vs baseline: 1.2657x; 1.2657x over previous
"""MoE (top-2 of 8 experts, d=1024) — expert-parallel Bass kernel for 8 trn2 cores.

Strategy (per sharding_hint "Expert-parallel"): shard W1/W2/b1/b2 along the
expert axis (expert e -> core e). The host computes the gate scores and top-2
assignment (0.2% of model FLOPs, deterministic) to build the dispatch: each
core receives exactly the tokens routed to its expert (padded to capacity C,
transposed and chunk-major so every HBM block streams sequentially). Each core
computes   yT = relu(W1^T xT + b1)^T W2 + b2   entirely in bf16 (PE runs bf16
at the same 1 cycle/row as float32r, but HBM traffic halves; max rel err vs
the fp32 reference is ~3e-3, well inside the 2e-2 gate). The host applies the
per-token combine weight and scatter-adds the two expert contributions.

Device-side details:
 - software-pipelined chunks: chunk s's first-layer matmuls interleave with
   chunk s-1's second-layer matmuls at 128-column granularity, so the PE
   never waits on the relu tail of a chunk
 - per-kc split DMAs so the first matmul waits on its first weight slice,
   not the whole 4MB
 - layer-1 bias+relu on DVE (tensor_scalar add+max from PSUM), layer-2 bias
   on the Act engine — splits the elementwise load across engines
 - all activations, weights and outputs bf16; PSUM accumulation fp32
"""

import numpy as np
import ml_dtypes

import concourse.bass as bass
import concourse.mybir as mybir
import concourse.tile as tile
from concourse import bacc
from concourse.bass_utils import run_bass_kernel_spmd

# Problem shapes (hardcoded per contract)
D = 1024  # d_model == d_hidden
N_EXPERTS = 8
TOP_K = 2
N_CORES = 8
B, T = 4, 2048
N_TOKENS = B * T

F32 = mybir.dt.float32
BF16 = mybir.dt.bfloat16
BF = ml_dtypes.bfloat16
KC = D // 128  # contraction chunks (8)
MC = D // 128  # output-feature chunks (8)
NT = 512      # tokens per matmul (moving free dim; one PSUM bank fp32)
CGRAIN = 128  # capacity granularity


def chunk_sizes(C):
    assert C % CGRAIN == 0
    sizes = [NT] * (C // NT)
    if C % NT:
        sizes.append(C % NT)
    return sizes


def build_moe_expert_kernel(C: int, repeat: int = 1, split_w: int = 8,
                            split_x: bool = True) -> bacc.Bacc:
    """One-expert MLP kernel: yT = relu(x@W1+b1)@W2 + b2, chunk-major bf16.

    DRAM inputs: xTb [nb, D, NT] (+ xTt [D, tail] if C%NT), w1 [D, D],
    b1 [D], w2 [D, D], b2 [D].  Outputs: yTb [nb, D, NT] (+ yTt [D, tail]).
    `repeat` wraps the computation in a hardware loop (slope-based HW timing).
    """
    sizes = chunk_sizes(C)
    nb = sum(1 for s in sizes if s == NT)
    tail = C % NT

    nc = bacc.Bacc("TRN2", target_bir_lowering=False, debug=False,
                   num_devices=N_CORES)

    xTb = nc.dram_tensor("xTb", [nb, D, NT], BF16, kind="ExternalInput")
    w1 = nc.dram_tensor("w1", [D, D], BF16, kind="ExternalInput")
    b1 = nc.dram_tensor("b1", [D], F32, kind="ExternalInput")
    w2 = nc.dram_tensor("w2", [D, D], BF16, kind="ExternalInput")
    b2 = nc.dram_tensor("b2", [D], F32, kind="ExternalInput")
    yTb = nc.dram_tensor("yTb", [nb, D, NT], BF16, kind="ExternalOutput")
    if tail:
        xTt = nc.dram_tensor("xTt", [D, tail], BF16, kind="ExternalInput")
        yTt = nc.dram_tensor("yTt", [D, tail], BF16, kind="ExternalOutput")

    # DRAM views: partition-dim-first tilings (chunk blocks are contiguous)
    xTb_v = xTb.ap().rearrange("n (kc kp) t -> n kp kc t", kc=KC)
    w1_v = w1.ap().rearrange("(kc kp) m -> kp kc m", kc=KC)      # [128, KC, D]
    w2_v = w2.ap().rearrange("(kc kp) m -> kp kc m", kc=KC)
    b1_v = b1.ap().rearrange("(mc mp) -> mp mc", mc=MC)          # [128, MC]
    b2_v = b2.ap().rearrange("(mc mp) -> mp mc", mc=MC)
    yTb_v = yTb.ap().rearrange("n (mc mp) t -> n mp mc t", mc=MC)
    if tail:
        xTt_v = xTt.ap().rearrange("(kc kp) t -> kp kc t", kc=KC)
        yTt_v = yTt.ap().rearrange("(mc mp) t -> mp mc t", mc=MC)

    def x_view(n):
        return xTb_v[n] if sizes[n] == NT else xTt_v

    def y_view(n):
        return yTb_v[n] if sizes[n] == NT else yTt_v

    with tile.TileContext(nc) as tc:
        with (
            tc.tile_pool(name="weights", bufs=1) as wpool,
            tc.tile_pool(name="consts", bufs=1) as cpool,
            tc.tile_pool(name="xin", bufs=3) as xpool,
            tc.tile_pool(name="hmid", bufs=2) as hpool,
            tc.tile_pool(name="yout", bufs=2) as ypool,
            tc.tile_pool(name="ph", bufs=3, space="PSUM") as phpool,
            tc.tile_pool(name="py", bufs=3, space="PSUM") as pypool,
        ):
            from contextlib import nullcontext
            loop_cm = (
                tc.For_i(0, repeat, 1,
                         hint_engines=(mybir.EngineType.PE,
                                       mybir.EngineType.Activation,
                                       mybir.EngineType.DVE,
                                       mybir.EngineType.SP))
                if repeat > 1 else nullcontext()
            )
            with loop_cm:
                w1_sb = wpool.tile([128, KC, D], BF16, tag="w1")
                w2_sb = wpool.tile([128, KC, D], BF16, tag="w2")
                b1_sb = cpool.tile([128, MC], F32, tag="b1")
                b2_sb = cpool.tile([128, MC], F32, tag="b2")

                offs = [sum(sizes[:i]) for i in range(len(sizes))]
                nchk = len(sizes)

                def emit_x_dma(n, x_sb):
                    sz = sizes[n]
                    if split_x:
                        for kc in range(KC):
                            nc.sync.dma_start(x_sb[:, kc, :sz],
                                              x_view(n)[:, kc, :])
                    else:
                        nc.sync.dma_start(x_sb[:, :, :sz], x_view(n))

                def emit_h_mc(n, mc, x_sb, h_sb):
                    sz = sizes[n]
                    ph = phpool.tile([128, NT], F32, tag="ph")
                    for kc in range(KC):
                        nc.tensor.matmul(
                            ph[:, :sz],
                            w1_sb[:, kc, bass.ts(mc, 128)],
                            x_sb[:, kc, :sz],
                            start=(kc == 0), stop=(kc == KC - 1),
                        )
                    # h = relu(ph + b1)   (DVE, PSUM -> SBUF bf16)
                    nc.vector.tensor_scalar(
                        h_sb[:, mc, :sz], ph[:, :sz],
                        b1_sb[:, mc:mc + 1], 0.0,
                        mybir.AluOpType.add, mybir.AluOpType.max,
                    )

                def emit_y_mc(n, mc, h_sb, y_sb):
                    sz = sizes[n]
                    py = pypool.tile([128, NT], F32, tag="py")
                    for kc in range(KC):
                        nc.tensor.matmul(
                            py[:, :sz],
                            w2_sb[:, kc, bass.ts(mc, 128)],
                            h_sb[:, kc, :sz],
                            start=(kc == 0), stop=(kc == KC - 1),
                        )
                    # y = py + b2   (Act engine, PSUM -> SBUF bf16)
                    nc.scalar.activation(
                        y_sb[:, mc, :sz], py[:, :sz],
                        mybir.ActivationFunctionType.Identity,
                        bias=b2_sb[:, mc:mc + 1],
                    )
                    nc.sync.dma_start(y_view(n)[:, mc, :],
                                      y_sb[:, mc, :sz])

                def emit_w2():
                    # w2 is first needed by chunk 0's y-phase; keep its DMA
                    # off the queue while w1/x0 races the PE.
                    for i in range(split_w):
                        ks = slice(i * (KC // split_w), (i + 1) * (KC // split_w))
                        nc.sync.dma_start(w2_sb[:, ks, :], w2_v[:, ks, :])
                    nc.sync.dma_start(b2_sb[:], b2_v)

                # Prologue: per-kc interleaved w1/x0 DMAs so the first matmul
                # waits only on its own slices.
                x0 = xpool.tile([128, KC, NT], BF16, tag="x")
                x_tiles = {0: x0}
                g = KC // split_w
                for i in range(split_w):
                    ks = slice(i * g, (i + 1) * g)
                    nc.sync.dma_start(w1_sb[:, ks, :], w1_v[:, ks, :])
                    if split_x:
                        for kc in range(i * g, (i + 1) * g):
                            nc.sync.dma_start(x_tiles[0][:, kc, :sizes[0]],
                                              x_view(0)[:, kc, :])
                if not split_x:
                    nc.sync.dma_start(x_tiles[0][:, :, :sizes[0]], x_view(0))
                nc.sync.dma_start(b1_sb[:], b1_v)

                # Software pipeline: stage s emits h-phase(s) interleaved with
                # y-phase(s-1) at mc granularity.
                h_tiles = {}
                y_tiles = {}
                for s in range(nchk + 1):
                    if s + 1 < nchk:  # prefetch x for next stage
                        xt = xpool.tile([128, KC, NT], BF16, tag="x")
                        emit_x_dma(s + 1, xt)
                        x_tiles[s + 1] = xt
                    if s < nchk:
                        h_tiles[s] = hpool.tile([128, KC, NT], BF16,
                                                tag="h", name=f"hs{s}")
                    if s > 0:
                        y_tiles[s - 1] = ypool.tile([128, MC, NT], BF16,
                                                    tag="y", name=f"ys{s}")
                    for mc in range(MC):
                        if s < nchk:
                            emit_h_mc(s, mc, x_tiles[s], h_tiles[s])
                        if s > 0:
                            emit_y_mc(s - 1, mc, h_tiles[s - 1],
                                      y_tiles[s - 1])
                    if s == 0:
                        emit_w2()
                    x_tiles.pop(s - 1, None)
                    h_tiles.pop(s - 2, None)
                    y_tiles.pop(s - 2, None)

    nc.compile()
    return nc


_NC_CACHE: dict = {}


def _get_kernel(C: int, repeat: int = 1, **opts) -> bacc.Bacc:
    key = (C, repeat, tuple(sorted(opts.items())))
    if key not in _NC_CACHE:
        _NC_CACHE[key] = build_moe_expert_kernel(C, repeat, **opts)
    return _NC_CACHE[key]


def dispatch(x, W_gate, b_gate):
    """Host-side gate + top-2 dispatch plan. Returns (xf, ids, wts, C)."""
    xf = np.ascontiguousarray(np.asarray(x).reshape(-1, D), dtype=np.float32)
    scores = xf @ np.asarray(W_gate, np.float32) + np.asarray(b_gate, np.float32)
    # top-2 expert ids per token (order irrelevant: contributions are summed)
    top2 = np.argpartition(scores, N_EXPERTS - TOP_K, axis=1)[:, -TOP_K:]
    ids, wts = [], []
    for e in range(N_EXPERTS):
        tok = np.nonzero((top2 == e).any(axis=1))[0]
        ids.append(tok)
        wts.append(scores[tok, e])
    max_cnt = max(len(t) for t in ids)
    C = ((max_cnt + CGRAIN - 1) // CGRAIN) * CGRAIN
    C = max(C, NT)  # at least one full chunk
    return xf, ids, wts, C


def make_in_maps(parts, xf, ids, wts, C):
    """Build per-core input dicts (chunk-major bf16 xT blocks)."""
    W1, b1, W2, b2 = parts
    sizes = chunk_sizes(C)
    nb = sum(1 for s in sizes if s == NT)
    tail = C % NT
    in_maps = []
    for e in range(N_EXPERTS):
        cnt = len(ids[e])
        xTe = np.zeros((D, C), BF)
        xTe[:, :cnt] = xf[ids[e]].T.astype(BF)
        xb = np.ascontiguousarray(
            xTe[:, :nb * NT].reshape(D, nb, NT).transpose(1, 0, 2))
        m = {
            "xTb": xb,
            "w1": np.ascontiguousarray(W1[e].astype(BF)),
            "b1": np.asarray(b1[e], np.float32),
            "w2": np.ascontiguousarray(W2[e].astype(BF)),
            "b2": np.asarray(b2[e], np.float32),
        }
        if tail:
            m["xTt"] = np.ascontiguousarray(xTe[:, nb * NT:])
        in_maps.append(m)
    return in_maps


def kernel(x, W_gate, b_gate, W1, b1, W2, b2):
    xf, ids, wts, C = dispatch(x, W_gate, b_gate)
    nc = _get_kernel(C)

    W1 = np.asarray(W1, np.float32)
    W2 = np.asarray(W2, np.float32)
    b1 = np.asarray(b1, np.float32)
    b2 = np.asarray(b2, np.float32)
    in_maps = make_in_maps((W1, b1, W2, b2), xf, ids, wts, C)

    res = run_bass_kernel_spmd(nc, in_maps, core_ids=list(range(N_CORES)))

    sizes = chunk_sizes(C)
    nb = sum(1 for s in sizes if s == NT)
    tail = C % NT
    out = np.zeros((N_TOKENS, D), np.float32)
    for e in range(N_EXPERTS):
        cnt = len(ids[e])
        r = res.results[e]
        yTe = r["yTb"].transpose(1, 0, 2).reshape(D, nb * NT)
        if tail:
            yTe = np.concatenate([yTe, r["yTt"]], axis=1)
        # combine: per-token gate weight applied host-side in fp32
        out[ids[e]] += yTe.T[:cnt].astype(np.float32) * wts[e][:, None]
    return out.reshape(B, T, D)


# revision 11
# speedup vs baseline: 1.3785x; 1.0892x over previous
"""MoE (top-2 of 8 experts, d=1024) — expert-parallel Bass kernel for 8 trn2 cores.

Strategy (per sharding_hint "Expert-parallel"): shard W1/W2/b1/b2 along the
expert axis (expert e -> core e). The host computes the gate scores and top-2
assignment (0.2% of model FLOPs, deterministic) to build the dispatch: each
core receives exactly the tokens routed to its expert (padded to capacity C).
Each core computes   yT = relu(W1^T xT + b1)^T W2 + b2   entirely in bf16
(the PE runs bf16 at the same 1 cycle/row as float32r, but HBM traffic
halves; max rel err vs the fp32 reference is ~3e-3, well inside the 2e-2
gate). The host applies the per-token combine weight and scatter-adds the two
expert contributions.

Device-side details:
 - all DRAM tensors are packed 128-partition-major so every DMA moves 8-16KB
   contiguous rows: ~14 DMA instructions / ~1.8K descriptors per iteration
   (the naive row-major layouts cost ~98 instructions / ~12.5K descriptors,
   which stalls the descriptor-generation path on real hardware)
 - software-pipelined chunks: chunk s's first-layer matmuls interleave with
   chunk s-1's second-layer matmuls at 128-column granularity, so the PE
   never waits on the relu tail of a chunk
 - the short tail chunk is processed FIRST so the last chunk's second-layer
   phase is a full 512 tokens — a wide window that hides the next
   iteration's W1 reload (wpool has a single buffer)
 - layer-1 bias+relu on DVE (tensor_scalar add+max from PSUM), layer-2 bias
   on the Act engine — splits the elementwise load across engines
 - all activations, weights and outputs bf16; PSUM accumulation fp32
"""

import numpy as np
import ml_dtypes

import concourse.bass as bass
import concourse.mybir as mybir
import concourse.tile as tile
from concourse import bacc
from concourse.bass_utils import run_bass_kernel_spmd

# Problem shapes (hardcoded per contract)
D = 1024  # d_model == d_hidden
N_EXPERTS = 8
TOP_K = 2
N_CORES = 8
B, T = 4, 2048
N_TOKENS = B * T

F32 = mybir.dt.float32
BF16 = mybir.dt.bfloat16
BF = ml_dtypes.bfloat16
KC = D // 128  # contraction chunks (8)
MC = D // 128  # output-feature chunks (8)
NT = 512      # tokens per matmul (moving free dim; one PSUM bank fp32)
CGRAIN = 128  # capacity granularity


def chunk_sizes(C):
    assert C % CGRAIN == 0
    sizes = [NT] * (C // NT)
    if C % NT:
        sizes.append(C % NT)
    return sizes


def build_moe_expert_kernel(C: int, repeat: int = 1, split_w: int = 2,
                            hoist_in: bool = False,
                            unroll: int = 1) -> bacc.Bacc:
    """One-expert MLP kernel: yT = relu(x@W1+b1)@W2 + b2, chunk-major bf16.

    DRAM inputs (all packed partition-major):
      xTb [nb, 128, KC, NT] (+ xTt [128, KC, tail] if C%NT),
      w1 [128, KC, D], b1 [128, MC], w2 [128, KC, D], b2 [128, MC].
    Outputs: yTb [nb, 128, MC, NT] (+ yTt [128, MC, tail]).
    `repeat` wraps the computation in a hardware loop (slope-based HW timing).
    """
    sizes = chunk_sizes(C)
    nb = sum(1 for s in sizes if s == NT)
    tail = C % NT
    nchk = len(sizes)
    # process the tail chunk first (see module docstring)
    order = ([nchk - 1] + list(range(nchk - 1))) if tail else list(range(nchk))

    nc = bacc.Bacc("TRN2", target_bir_lowering=False, debug=False,
                   num_devices=N_CORES)

    xTb = nc.dram_tensor("xTb", [nb, 128, KC, NT], BF16, kind="ExternalInput")
    w1 = nc.dram_tensor("w1", [128, KC, D], BF16, kind="ExternalInput")
    b1 = nc.dram_tensor("b1", [128, MC], F32, kind="ExternalInput")
    w2 = nc.dram_tensor("w2", [128, KC, D], BF16, kind="ExternalInput")
    b2 = nc.dram_tensor("b2", [128, MC], F32, kind="ExternalInput")
    yTb = nc.dram_tensor("yTb", [nb, 128, MC, NT], BF16,
                         kind="ExternalOutput")
    if tail:
        xTt = nc.dram_tensor("xTt", [128, KC, tail], BF16,
                             kind="ExternalInput")
        yTt = nc.dram_tensor("yTt", [128, MC, tail], BF16,
                             kind="ExternalOutput")

    def x_view(n):
        return xTb.ap()[n] if sizes[n] == NT else xTt.ap()

    def y_view(n):
        return yTb.ap()[n] if sizes[n] == NT else yTt.ap()

    with tile.TileContext(nc) as tc:
        with (
            tc.tile_pool(name="weights", bufs=1) as wpool,
            tc.tile_pool(name="consts", bufs=1) as cpool,
            tc.tile_pool(name="xin",
                         bufs=(nchk if hoist_in else 3)) as xpool,
            tc.tile_pool(name="hmid", bufs=2) as hpool,
            tc.tile_pool(name="yout", bufs=2) as ypool,
            tc.tile_pool(name="ph", bufs=3, space="PSUM") as phpool,
            tc.tile_pool(name="py", bufs=3, space="PSUM") as pypool,
        ):
            from contextlib import nullcontext
            loop_cm = (
                tc.For_i(0, repeat, 1,
                         hint_engines=(mybir.EngineType.PE,
                                       mybir.EngineType.Activation,
                                       mybir.EngineType.DVE,
                                       mybir.EngineType.SP))
                if repeat > 1 else nullcontext()
            )
            state: dict = {}

            def alloc_weights():
                state["w1_sb"] = wpool.tile([128, KC, D], BF16, tag="w1",
                                            name="w1_sb")
                state["w2_sb"] = wpool.tile([128, KC, D], BF16, tag="w2",
                                            name="w2_sb")
                state["b1_sb"] = cpool.tile([128, MC], F32, tag="b1",
                                            name="b1_sb")
                state["b2_sb"] = cpool.tile([128, MC], F32, tag="b2",
                                            name="b2_sb")

            def emit_h_mc(n, mc, x_sb, h_sb):
                sz = sizes[n]
                ph = phpool.tile([128, NT], F32, tag="ph", name="ph")
                for kc in range(KC):
                    nc.tensor.matmul(
                        ph[:, :sz],
                        state["w1_sb"][:, kc, bass.ts(mc, 128)],
                        x_sb[:, kc, :sz],
                        start=(kc == 0), stop=(kc == KC - 1),
                    )
                # h = relu(ph + b1)   (DVE, PSUM -> SBUF bf16)
                nc.vector.tensor_scalar(
                    h_sb[:, mc, :sz], ph[:, :sz],
                    state["b1_sb"][:, mc:mc + 1], 0.0,
                    mybir.AluOpType.add, mybir.AluOpType.max,
                )

            def emit_y_mc(n, mc, h_sb, y_sb):
                sz = sizes[n]
                py = pypool.tile([128, NT], F32, tag="py", name="py")
                for kc in range(KC):
                    nc.tensor.matmul(
                        py[:, :sz],
                        state["w2_sb"][:, kc, bass.ts(mc, 128)],
                        h_sb[:, kc, :sz],
                        start=(kc == 0), stop=(kc == KC - 1),
                    )
                # y = py + b2   (Act engine, PSUM -> SBUF bf16)
                nc.scalar.activation(
                    y_sb[:, mc, :sz], py[:, :sz],
                    mybir.ActivationFunctionType.Identity,
                    bias=state["b2_sb"][:, mc:mc + 1],
                )

            def emit_w2():
                # w2 is first needed by the first chunk's y-phase; keep its
                # DMA off the queue while w1/x0 races the PE.
                g = KC // split_w
                for i in range(split_w):
                    ks = slice(i * g, (i + 1) * g)
                    nc.sync.dma_start(state["w2_sb"][:, ks, :],
                                      w2.ap()[:, ks, :])
                nc.sync.dma_start(state["b2_sb"][:], b2.ap())

            def emit_prologue(x_tiles):
                # Interleaved w1/x0 DMAs so the first matmuls wait only on
                # their own slices.
                g = KC // split_w
                n0 = order[0]
                for i in range(split_w):
                    ks = slice(i * g, (i + 1) * g)
                    nc.sync.dma_start(state["w1_sb"][:, ks, :],
                                      w1.ap()[:, ks, :])
                    nc.sync.dma_start(x_tiles[n0][:, ks, :sizes[n0]],
                                      x_view(n0)[:, ks, :])
                nc.sync.dma_start(state["b1_sb"][:], b1.ap())

            def emit_pipeline(x_tiles, prefetch):
                # Software pipeline: stage s emits h-phase(s) interleaved
                # with y-phase(s-1) at mc granularity.
                h_tiles = {}
                y_tiles = {}
                for si in range(nchk + 1):
                    s = order[si] if si < nchk else None
                    p = order[si - 1] if si > 0 else None
                    if prefetch and si + 1 < nchk:  # prefetch next stage's x
                        nxt = order[si + 1]
                        xt = xpool.tile([128, KC, NT], BF16, tag="x",
                                        name=f"xs{si + 1}")
                        nc.sync.dma_start(xt[:, :, :sizes[nxt]], x_view(nxt))
                        x_tiles[nxt] = xt
                    if s is not None:
                        h_tiles[s] = hpool.tile([128, KC, NT], BF16,
                                                tag="h", name=f"hs{si}")
                    if p is not None:
                        y_tiles[p] = ypool.tile([128, MC, NT], BF16,
                                                tag="y", name=f"ys{si}")
                    for mc in range(MC):
                        if s is not None:
                            emit_h_mc(s, mc, x_tiles[s], h_tiles[s])
                        if p is not None:
                            emit_y_mc(p, mc, h_tiles[p], y_tiles[p])
                    if p is not None:
                        # single writeback per chunk: 128 contiguous 8KB rows
                        nc.sync.dma_start(y_view(p)[:, :, :],
                                          y_tiles[p][:, :, :sizes[p]])
                    if si == 0 and not hoist_in:
                        emit_w2()
                    if prefetch and si >= 1:
                        x_tiles.pop(order[si - 1], None)
                    if si >= 2:
                        h_tiles.pop(order[si - 2], None)
                        y_tiles.pop(order[si - 2], None)

            if hoist_in:
                # All input DMAs once, outside the repeat loop: the loop
                # measures pure compute + output writeback.
                alloc_weights()
                nc.sync.dma_start(state["w1_sb"][:], w1.ap())
                nc.sync.dma_start(state["b1_sb"][:], b1.ap())
                emit_w2()
                x_tiles = {}
                for n in range(nchk):
                    x_tiles[n] = xpool.tile([128, KC, NT], BF16, tag="x",
                                            name=f"xh{n}")
                    nc.sync.dma_start(x_tiles[n][:, :, :sizes[n]], x_view(n))
                with loop_cm:
                    emit_pipeline(x_tiles, prefetch=False)
            else:
                with loop_cm:
                    for _ in range(unroll):
                        alloc_weights()
                        n0 = order[0]
                        x0 = xpool.tile([128, KC, NT], BF16, tag="x",
                                        name="x0")
                        x_tiles = {n0: x0}
                        emit_prologue(x_tiles)
                        emit_pipeline(x_tiles, prefetch=True)

    nc.compile()
    return nc


_NC_CACHE: dict = {}


def _get_kernel(C: int, repeat: int = 1, **opts) -> bacc.Bacc:
    key = (C, repeat, tuple(sorted(opts.items())))
    if key not in _NC_CACHE:
        _NC_CACHE[key] = build_moe_expert_kernel(C, repeat, **opts)
    return _NC_CACHE[key]


def dispatch(x, W_gate, b_gate):
    """Host-side gate + top-2 dispatch plan. Returns (xf, ids, wts, C)."""
    xf = np.ascontiguousarray(np.asarray(x).reshape(-1, D), dtype=np.float32)
    scores = xf @ np.asarray(W_gate, np.float32) + np.asarray(b_gate, np.float32)
    # top-2 expert ids per token (order irrelevant: contributions are summed)
    top2 = np.argpartition(scores, N_EXPERTS - TOP_K, axis=1)[:, -TOP_K:]
    ids, wts = [], []
    for e in range(N_EXPERTS):
        tok = np.nonzero((top2 == e).any(axis=1))[0]
        ids.append(tok)
        wts.append(scores[tok, e])
    max_cnt = max(len(t) for t in ids)
    C = ((max_cnt + CGRAIN - 1) // CGRAIN) * CGRAIN
    C = max(C, NT)  # at least one full chunk
    return xf, ids, wts, C


def pack_rows(a):
    """[D, n] row-major (kc kp) -> [128, KC, n] partition-major."""
    return np.ascontiguousarray(a.reshape(KC, 128, -1).transpose(1, 0, 2))


def make_in_maps(parts, xf, ids, wts, C):
    """Build per-core input dicts (packed partition-major bf16 blocks)."""
    W1, b1, W2, b2 = parts
    sizes = chunk_sizes(C)
    nb = sum(1 for s in sizes if s == NT)
    tail = C % NT
    in_maps = []
    for e in range(N_EXPERTS):
        cnt = len(ids[e])
        xTe = np.zeros((D, C), BF)
        xTe[:, :cnt] = xf[ids[e]].T.astype(BF)
        xp = pack_rows(xTe)  # [128, KC, C]
        xb = np.ascontiguousarray(
            xp[:, :, :nb * NT].reshape(128, KC, nb, NT).transpose(2, 0, 1, 3))
        m = {
            "xTb": xb,
            "w1": pack_rows(W1[e].astype(BF)),
            "b1": np.ascontiguousarray(
                np.asarray(b1[e], np.float32).reshape(MC, 128).T),
            "w2": pack_rows(W2[e].astype(BF)),
            "b2": np.ascontiguousarray(
                np.asarray(b2[e], np.float32).reshape(MC, 128).T),
        }
        if tail:
            m["xTt"] = np.ascontiguousarray(xp[:, :, nb * NT:])
        in_maps.append(m)
    return in_maps


def kernel(x, W_gate, b_gate, W1, b1, W2, b2):
    xf, ids, wts, C = dispatch(x, W_gate, b_gate)
    nc = _get_kernel(C)

    W1 = np.asarray(W1, np.float32)
    W2 = np.asarray(W2, np.float32)
    b1 = np.asarray(b1, np.float32)
    b2 = np.asarray(b2, np.float32)
    in_maps = make_in_maps((W1, b1, W2, b2), xf, ids, wts, C)

    res = run_bass_kernel_spmd(nc, in_maps, core_ids=list(range(N_CORES)))

    sizes = chunk_sizes(C)
    nb = sum(1 for s in sizes if s == NT)
    tail = C % NT
    out = np.zeros((N_TOKENS, D), np.float32)
    for e in range(N_EXPERTS):
        cnt = len(ids[e])
        r = res.results[e]
        # yTb [nb, 128, MC, NT] -> yT [(mc mp), nb*NT]
        yTe = r["yTb"].transpose(2, 1, 0, 3).reshape(D, nb * NT)
        if tail:
            yTt = r["yTt"].transpose(1, 0, 2).reshape(D, tail)
            yTe = np.concatenate([yTe, yTt], axis=1)
        # combine: per-token gate weight applied host-side in fp32
        out[ids[e]] += yTe.T[:cnt].astype(np.float32) * wts[e][:, None]
    return out.reshape(B, T, D)


# revision 16
# speedup vs baseline: 1.6267x; 1.1800x over previous
"""MoE (top-2 of 8 experts, d=1024) — load-balanced hidden-split Bass kernel
for 8 trn2 cores.

Sharding (refinement of the sharding_hint's expert-parallel scheme): each
expert's MLP is split in half along the HIDDEN dimension (512 units each),
giving 16 shards. Shards are paired onto cores so that a big expert always
shares a core with a small expert: experts sorted by routed-token count,
pair p = (desc[p], desc[7-p]); core 2p takes hidden-half 0 of both, core
2p+1 takes hidden-half 1. Every core runs an identical program with token
capacities (CA, CB) = (padded max big count, padded max small count) — about
(2304+2048) slots instead of the 2*2304 a pure expert-parallel layout needs,
a ~9% PE-cycle cut. Partial y outputs (each half contributes a full-d
partial sum) are combined on the host, which also applies the per-token
top-2 gate weights (the y = yh0 + yh1 sum and the combine are both linear).

Device kernel (per core): for shard S in {A, B}:
    yS_partial = relu(xS @ W1S + b1S) @ W2S (+ b2 on half-0 cores only)
entirely in bf16 (PE runs bf16 at 1 cycle/row like float32r, but HBM
traffic halves; max rel err vs the fp32 reference ~4e-3, inside the 2e-2
gate). fp32 PSUM accumulation.

Device-side details:
 - all DRAM tensors packed 128-partition-major so every DMA moves 8-16KB
   contiguous rows (tiny descriptor counts)
 - software-pipelined chunks across both shards: chunk s's layer-1 matmuls
   interleave with chunk s-1's layer-2 matmuls at 128-column granularity
 - tail chunks processed first within each shard so the iteration ends on a
   full 512-token layer-2 phase, hiding the next iteration's weight reloads
 - layer-1 bias+relu on DVE, layer-2 bias on Act — splits elementwise load
"""

import numpy as np
import ml_dtypes

import concourse.bass as bass
import concourse.mybir as mybir
import concourse.tile as tile
from concourse import bacc
from concourse.bass_utils import run_bass_kernel_spmd

# Problem shapes (hardcoded per contract)
D = 1024   # d_model == d_hidden
HD = 512   # hidden half per shard
N_EXPERTS = 8
TOP_K = 2
N_CORES = 8
B, T = 4, 2048
N_TOKENS = B * T

F32 = mybir.dt.float32
BF16 = mybir.dt.bfloat16
BF = ml_dtypes.bfloat16
KC = D // 128    # layer-1 contraction chunks (8)
KH = HD // 128   # layer-2 contraction chunks (4) == layer-1 output chunks
MC = D // 128    # layer-2 output chunks (8)
NT = 512         # tokens per matmul (moving free dim; one PSUM bank fp32)
CGRAIN = 128     # capacity granularity


def chunk_list(C):
    """Chunk sizes in processing order: tail (if any) first, then 512s."""
    assert C % CGRAIN == 0 and C >= NT
    sizes = [NT] * (C // NT)
    if C % NT:
        sizes = [C % NT] + sizes
    return sizes


def build_moe_expert_kernel(C, repeat: int = 1, split_w: int = 2,
                            hoist_in: bool = False,
                            unroll: int = 1) -> bacc.Bacc:
    """Two half-expert shards A (cap CA) and B (cap CB), C = (CA, CB).

    DRAM inputs (packed partition-major, bf16 except biases):
      xA [nA, 128, KC, NT] (+ xAt [128, KC, tailA]), same for B
      wA1 [128, KC, HD], wA2 [128, KH, D], bA1 [128, KH], bA2 [128, MC]
      (same for B)
    Outputs: yA [nA, 128, MC, NT] (+ yAt), yB likewise (partial sums).
    `repeat` wraps the body in a hardware loop (slope-based HW timing).
    """
    CA, CB = C[0], C[1]
    nc = bacc.Bacc("TRN2", target_bir_lowering=False, debug=False,
                   num_devices=N_CORES)

    shards = []
    for sname, cap in (("A", CA), ("B", CB)):
        sizes = chunk_list(cap)
        nfull = sum(1 for s in sizes if s == NT)
        tail = cap % NT
        sd = {
            "name": sname, "sizes": sizes, "tail": tail, "nfull": nfull,
            "x": nc.dram_tensor(f"x{sname}", [nfull, 128, KC, NT], BF16,
                                kind="ExternalInput"),
            "w1": nc.dram_tensor(f"w{sname}1", [128, KC, HD], BF16,
                                 kind="ExternalInput"),
            "b1": nc.dram_tensor(f"b{sname}1", [128, KH], F32,
                                 kind="ExternalInput"),
            "w2": nc.dram_tensor(f"w{sname}2", [128, KH, D], BF16,
                                 kind="ExternalInput"),
            "b2": nc.dram_tensor(f"b{sname}2", [128, MC], F32,
                                 kind="ExternalInput"),
            "y": nc.dram_tensor(f"y{sname}", [nfull, 128, MC, NT], BF16,
                                kind="ExternalOutput"),
        }
        if tail:
            sd["xt"] = nc.dram_tensor(f"x{sname}t", [128, KC, tail], BF16,
                                      kind="ExternalInput")
            sd["yt"] = nc.dram_tensor(f"y{sname}t", [128, MC, tail], BF16,
                                      kind="ExternalOutput")
        shards.append(sd)

    def x_view(sd, n):
        # chunk n in processing order; tail (if any) is chunk 0
        if sd["tail"]:
            return sd["xt"].ap() if n == 0 else sd["x"].ap()[n - 1]
        return sd["x"].ap()[n]

    def y_view(sd, n):
        if sd["tail"]:
            return sd["yt"].ap() if n == 0 else sd["y"].ap()[n - 1]
        return sd["y"].ap()[n]

    # pipeline stages: (shard, chunk) in processing order
    stages = [(sd, n) for sd in shards for n in range(len(sd["sizes"]))]
    nst = len(stages)

    with tile.TileContext(nc) as tc:
        with (
            tc.tile_pool(name="weights", bufs=1) as wpool,
            tc.tile_pool(name="consts", bufs=1) as cpool,
            tc.tile_pool(name="xin",
                         bufs=(nst if hoist_in else 3)) as xpool,
            tc.tile_pool(name="hmid", bufs=2) as hpool,
            tc.tile_pool(name="yout", bufs=2) as ypool,
            tc.tile_pool(name="ph", bufs=3, space="PSUM") as phpool,
            tc.tile_pool(name="py", bufs=3, space="PSUM") as pypool,
        ):
            from contextlib import nullcontext
            loop_cm = (
                tc.For_i(0, repeat, 1,
                         hint_engines=(mybir.EngineType.PE,
                                       mybir.EngineType.Activation,
                                       mybir.EngineType.DVE,
                                       mybir.EngineType.SP))
                if repeat > 1 else nullcontext()
            )
            state: dict = {}

            def alloc_tiles():
                for sd in shards:
                    s = sd["name"]
                    state[f"w1{s}"] = wpool.tile([128, KC, HD], BF16,
                                                 tag=f"w1{s}",
                                                 name=f"w1{s}_sb")
                    state[f"w2{s}"] = wpool.tile([128, KH, D], BF16,
                                                 tag=f"w2{s}",
                                                 name=f"w2{s}_sb")
                    state[f"b1{s}"] = cpool.tile([128, KH], F32,
                                                 tag=f"b1{s}",
                                                 name=f"b1{s}_sb")
                    state[f"b2{s}"] = cpool.tile([128, MC], F32,
                                                 tag=f"b2{s}",
                                                 name=f"b2{s}_sb")

            def emit_h_mc(sd, n, mc, x_sb, h_sb):
                sz = sd["sizes"][n]
                s = sd["name"]
                ph = phpool.tile([128, NT], F32, tag="ph", name="ph")
                for kc in range(KC):
                    nc.tensor.matmul(
                        ph[:, :sz],
                        state[f"w1{s}"][:, kc, bass.ts(mc, 128)],
                        x_sb[:, kc, :sz],
                        start=(kc == 0), stop=(kc == KC - 1),
                    )
                # h = relu(ph + b1)   (DVE, PSUM -> SBUF bf16)
                nc.vector.tensor_scalar(
                    h_sb[:, mc, :sz], ph[:, :sz],
                    state[f"b1{s}"][:, mc:mc + 1], 0.0,
                    mybir.AluOpType.add, mybir.AluOpType.max,
                )

            def emit_y_mc(sd, n, mc, h_sb, y_sb):
                sz = sd["sizes"][n]
                s = sd["name"]
                py = pypool.tile([128, NT], F32, tag="py", name="py")
                for kh in range(KH):
                    nc.tensor.matmul(
                        py[:, :sz],
                        state[f"w2{s}"][:, kh, bass.ts(mc, 128)],
                        h_sb[:, kh, :sz],
                        start=(kh == 0), stop=(kh == KH - 1),
                    )
                # y = py + b2   (Act engine, PSUM -> SBUF bf16)
                nc.scalar.activation(
                    y_sb[:, mc, :sz], py[:, :sz],
                    mybir.ActivationFunctionType.Identity,
                    bias=state[f"b2{s}"][:, mc:mc + 1],
                )

            def emit_w_dma(sd, which):
                s = sd["name"]
                if which == 1:
                    nc.sync.dma_start(state[f"w1{s}"][:], sd["w1"].ap())
                    nc.sync.dma_start(state[f"b1{s}"][:], sd["b1"].ap())
                else:
                    nc.sync.dma_start(state[f"w2{s}"][:], sd["w2"].ap())
                    nc.sync.dma_start(state[f"b2{s}"][:], sd["b2"].ap())

            def emit_prologue(x_tiles):
                # Interleaved wA1/x0 DMAs so the first matmuls wait only on
                # their own slices.
                sd0, n0 = stages[0]
                g = KC // split_w
                for i in range(split_w):
                    ks = slice(i * g, (i + 1) * g)
                    nc.sync.dma_start(state["w1A"][:, ks, :],
                                      sd0["w1"].ap()[:, ks, :])
                    nc.sync.dma_start(x_tiles[0][:, ks, :sd0["sizes"][n0]],
                                      x_view(sd0, n0)[:, ks, :])
                nc.sync.dma_start(state["b1A"][:], sd0["b1"].ap())

            def emit_pipeline(x_tiles, prefetch):
                h_tiles = {}
                y_tiles = {}
                for si in range(nst + 1):
                    cur = stages[si] if si < nst else None
                    prev = stages[si - 1] if si > 0 else None
                    if prefetch and si + 1 < nst:  # prefetch next stage's x
                        sdn, nn = stages[si + 1]
                        xt = xpool.tile([128, KC, NT], BF16, tag="x",
                                        name=f"xs{si + 1}")
                        nc.sync.dma_start(xt[:, :, :sdn["sizes"][nn]],
                                          x_view(sdn, nn))
                        x_tiles[si + 1] = xt
                    if cur is not None:
                        h_tiles[si] = hpool.tile([128, KH, NT], BF16,
                                                 tag="h", name=f"hs{si}")
                    if prev is not None:
                        y_tiles[si - 1] = ypool.tile([128, MC, NT], BF16,
                                                     tag="y", name=f"ys{si}")
                    for mc in range(MC):
                        if cur is not None and mc < KH:
                            emit_h_mc(cur[0], cur[1], mc, x_tiles[si],
                                      h_tiles[si])
                        if prev is not None:
                            emit_y_mc(prev[0], prev[1], mc, h_tiles[si - 1],
                                      y_tiles[si - 1])
                    if prev is not None:
                        # single writeback per chunk: 128 contiguous 8KB rows
                        sdp, np_ = prev
                        nc.sync.dma_start(
                            y_view(sdp, np_)[:, :, :],
                            y_tiles[si - 1][:, :, :sdp["sizes"][np_]])
                    if not hoist_in:
                        # spread remaining weight DMAs across early stages
                        if si == 0:
                            emit_w_dma(shards[0], 2)
                        elif si == 1:
                            emit_w_dma(shards[1], 1)
                        elif si == 2:
                            emit_w_dma(shards[1], 2)
                    if prefetch:
                        x_tiles.pop(si - 1, None)
                    h_tiles.pop(si - 2, None)
                    y_tiles.pop(si - 2, None)

            if hoist_in:
                alloc_tiles()
                for sd in shards:
                    emit_w_dma(sd, 1)
                    emit_w_dma(sd, 2)
                x_tiles = {}
                for si, (sd, n) in enumerate(stages):
                    x_tiles[si] = xpool.tile([128, KC, NT], BF16, tag="x",
                                             name=f"xh{si}")
                    nc.sync.dma_start(x_tiles[si][:, :, :sd["sizes"][n]],
                                      x_view(sd, n))
                with loop_cm:
                    emit_pipeline(x_tiles, prefetch=False)
            else:
                with loop_cm:
                    for _ in range(unroll):
                        alloc_tiles()
                        x0 = xpool.tile([128, KC, NT], BF16, tag="x",
                                        name="x0")
                        x_tiles = {0: x0}
                        emit_prologue(x_tiles)
                        emit_pipeline(x_tiles, prefetch=True)

    nc.compile()
    return nc


_NC_CACHE: dict = {}


def _get_kernel(C, repeat: int = 1, **opts) -> bacc.Bacc:
    key = (C, repeat, tuple(sorted(opts.items())))
    if key not in _NC_CACHE:
        _NC_CACHE[key] = build_moe_expert_kernel(C, repeat, **opts)
    return _NC_CACHE[key]


def _pad(n):
    return max(NT, ((n + CGRAIN - 1) // CGRAIN) * CGRAIN)


def dispatch(x, W_gate, b_gate):
    """Host-side gate + top-2 dispatch plan. Returns (xf, ids, wts, C)."""
    xf = np.ascontiguousarray(np.asarray(x).reshape(-1, D), dtype=np.float32)
    scores = xf @ np.asarray(W_gate, np.float32) + np.asarray(b_gate, np.float32)
    # top-2 expert ids per token (order irrelevant: contributions are summed)
    top2 = np.argpartition(scores, N_EXPERTS - TOP_K, axis=1)[:, -TOP_K:]
    ids, wts = [], []
    for e in range(N_EXPERTS):
        tok = np.nonzero((top2 == e).any(axis=1))[0]
        ids.append(tok)
        wts.append(scores[tok, e])
    counts = [len(t) for t in ids]
    order = list(np.argsort(-np.asarray(counts), kind="stable"))
    # pair p: (big, small) -> cores 2p (hidden half 0) and 2p+1 (half 1)
    pairs = [(int(order[p]), int(order[7 - p])) for p in range(4)]
    CA = _pad(max(counts[a] for a, _ in pairs))
    CB = _pad(max(counts[b] for _, b in pairs))
    return xf, ids, wts, (CA, CB, tuple(pairs))


def pack_rows(a):
    """[(kc kp), n] row-major -> [128, nkc, n] partition-major."""
    nkc = a.shape[0] // 128
    return np.ascontiguousarray(a.reshape(nkc, 128, -1).transpose(1, 0, 2))


def _pack_x(xTe, cap):
    """xT [D, cnt] -> packed chunk blocks (tail chunk first)."""
    Dd, cnt = xTe.shape
    xp = np.zeros((128, KC, cap), BF)
    xp[:, :, :cnt] = pack_rows(xTe)
    tail = cap % NT
    nfull = cap // NT
    xb = np.ascontiguousarray(
        xp[:, :, tail:].reshape(128, KC, nfull, NT).transpose(2, 0, 1, 3))
    xt = np.ascontiguousarray(xp[:, :, :tail]) if tail else None
    return xb, xt


def make_in_maps(parts, xf, ids, wts, C):
    """Build per-core input dicts (packed partition-major bf16 blocks)."""
    W1, b1, W2, b2 = parts
    CA, CB, pairs = C
    in_maps = []
    for p in range(4):
        for h in range(2):
            hs = slice(h * HD, (h + 1) * HD)
            m = {}
            for sname, cap, e in (("A", CA, pairs[p][0]),
                                  ("B", CB, pairs[p][1])):
                xTe = xf[ids[e]].T.astype(BF)
                xb, xt = _pack_x(xTe, cap)
                m[f"x{sname}"] = xb
                if xt is not None:
                    m[f"x{sname}t"] = xt
                m[f"w{sname}1"] = pack_rows(
                    np.asarray(W1[e][:, hs], np.float32).astype(BF))
                m[f"w{sname}2"] = pack_rows(
                    np.asarray(W2[e][hs, :], np.float32).astype(BF))
                m[f"b{sname}1"] = np.ascontiguousarray(
                    np.asarray(b1[e][hs], np.float32).reshape(KH, 128).T)
                b2v = (np.asarray(b2[e], np.float32) if h == 0
                       else np.zeros(D, np.float32))
                m[f"b{sname}2"] = np.ascontiguousarray(
                    b2v.reshape(MC, 128).T)
            in_maps.append(m)
    return in_maps


def _unpack_y(r, sname, cap):
    """packed y blocks -> yT [D, cap] fp32 (tail chunk first)."""
    tail = cap % NT
    nfull = cap // NT
    yb = r[f"y{sname}"].transpose(2, 1, 0, 3).reshape(D, nfull * NT)
    if tail:
        yt = r[f"y{sname}t"].transpose(1, 0, 2).reshape(D, tail)
        yb = np.concatenate([yt, yb], axis=1)
    return yb.astype(np.float32)


def kernel(x, W_gate, b_gate, W1, b1, W2, b2):
    xf, ids, wts, C = dispatch(x, W_gate, b_gate)
    CA, CB, pairs = C
    nc = _get_kernel(C)

    in_maps = make_in_maps((W1, b1, W2, b2), xf, ids, wts, C)
    res = run_bass_kernel_spmd(nc, in_maps, core_ids=list(range(N_CORES)))

    out = np.zeros((N_TOKENS, D), np.float32)
    for p in range(4):
        r0, r1 = res.results[2 * p], res.results[2 * p + 1]
        for sname, cap, e in (("A", CA, pairs[p][0]),
                              ("B", CB, pairs[p][1])):
            cnt = len(ids[e])
            yT = _unpack_y(r0, sname, cap) + _unpack_y(r1, sname, cap)
            out[ids[e]] += yT.T[:cnt] * wts[e][:, None]
    return out.reshape(B, T, D)
